# revision 2
# baseline (speedup 1.0000x reference)
"""Trainium2 Bass kernel v2 for nn_AttentionModel (greedy pointer-attention decode).

Architecture (per core, 128 batch items on partitions):
  Precompute:
    emb2 = emb + pref; projections via PE: gv_d [BS,N,256] (glimpse V),
    qdT_d / gkT_d / lk2T_d [256, BS*N] feature-major transposed slabs
    (lk2T has W_out folded: lK2 = emb2 @ (W_node[:,512:] @ W_out^T)).
    fixed2 = mean(emb2)@W_fixed + first@W_step[:256]; fixed2T columns.
    S_d [BS*N, 8*200] f32: full compat table S[b,p,h,n] = (1/sqrt(32)) *
    sum_e (fixed2[b]+qd[b,p])_{h,e} * gK[b,n,h,e]  -- per-item PE matmuls
    with 4-head row-group tiling.
  Per step:
    indirect-gather S row (b, prev_b) -> [128, 8, 200] compat (pre-scaled);
    mask+softmax (DVE/ACT); glimpse = sum_n attn*gV in two feature-halves,
    mult on GPSIMD/DVE split + segmented reduce on DVE, V partially
    SBUF-resident, rest streamed; glimpse transposed via PE -> grawT;
    logits = per-item PE matvec with lK2T chunk as the *stationary* operand
    (LDWEIGHTS-bandwidth bound) and grawT column as 1-wide moving operand,
    accumulated over the 2 feature-halves in PSUM; retranspose via PE with
    tanh fused into the PSUM->SBUF ACT copy; argmax via DVE max/max_index;
    visited mask as 0/-1e9 addend; log_softmax via ACT exp-accum/ln.
All math f32 (top-2 logit gaps down to 1.9e-6 forbid 16-bit anywhere).
"""
import numpy as np

import concourse.bass as bass
from concourse import bacc
import concourse.tile as tile
from concourse import mybir
from concourse.bass import IndirectOffsetOnAxis
from concourse.bass_utils import run_bass_kernel_spmd

dt = mybir.dt
F32 = dt.float32
AX = mybir.AxisListType
OP = mybir.AluOpType
ACTF = mybir.ActivationFunctionType

B, N, D, H = 1024, 200, 256, 8
d = D // H                      # 32
NCORES = 8
BS = B // NCORES                # 128 items per core
T = N - 1                       # 199 decode steps
START = 24
NEG = -1e9
ISD32 = float(np.float32(1.0 / np.sqrt(32.0)))
ISD256 = 0.0625                 # 1/sqrt(256)

RN = 120                        # gV nodes resident in SBUF
CH = 10                         # glimpse node-chunk
IC = 8                          # logits item-chunk
ROWT = BS * N // 128            # 200 row-tiles in precompute


def _build():
    nc = bacc.Bacc("TRN2", target_bir_lowering=False, debug=False)

    emb_in = nc.dram_tensor("embeddings", [BS, N, D], F32, kind="ExternalInput").ap()
    pref_in = nc.dram_tensor("pref_embed", [D], F32, kind="ExternalInput").ap()
    wnode_in = nc.dram_tensor("W_node", [D, 3 * D], F32, kind="ExternalInput").ap()
    wfix_in = nc.dram_tensor("W_fixed", [D, D], F32, kind="ExternalInput").ap()
    wstep_in = nc.dram_tensor("W_step", [2 * D, D], F32, kind="ExternalInput").ap()
    wout_in = nc.dram_tensor("W_out", [D, D], F32, kind="ExternalInput").ap()

    out = nc.dram_tensor("log_p", [BS, T * N], F32, kind="ExternalOutput").ap()

    emb2_d = nc.dram_tensor("emb2_d", [BS * N, D], F32).ap()
    gv_d = nc.dram_tensor("gv_d", [BS, N, D], F32).ap()
    qdT_d = nc.dram_tensor("qdT_d", [D, BS * N], F32).ap()
    gkT_d = nc.dram_tensor("gkT_d", [D, BS * N], F32).ap()
    lk2T_d = nc.dram_tensor("lk2T_d", [D, BS * N], F32).ap()
    S_d = nc.dram_tensor("S_d", [BS * N, H * N], F32).ap()

    with tile.TileContext(nc) as tc:
        with (
            tc.tile_pool(name="res", bufs=1) as res,          # persistent state
            tc.tile_pool(name="psmall", bufs=2, space="PSUM") as psmall,
        ):
            # ---------- persistent small state ----------
            ident = res.tile([128, 128], F32)
            io_c = res.tile([128, 128], dt.int32)
            nc.gpsimd.iota(io_c[:], pattern=[[1, 128]], channel_multiplier=0)
            io_r = res.tile([128, 1], dt.int32)
            nc.gpsimd.iota(io_r[:], pattern=[[0, 1]], channel_multiplier=1)
            id_i = res.tile([128, 128], dt.int32)
            nc.vector.tensor_tensor(id_i[:], io_c[:], io_r[:].broadcast_to([128, 128]), op=OP.is_equal)
            nc.vector.tensor_copy(ident[:], id_i[:])

            iota_n = res.tile([128, N], dt.int32)
            nc.gpsimd.iota(iota_n[:], pattern=[[1, N]], channel_multiplier=0)
            iota_row = res.tile([128, 1], dt.int32)     # b*N
            nc.gpsimd.iota(iota_row[:], pattern=[[0, 1]], channel_multiplier=N)

            amask = res.tile([128, N], F32)
            nc.vector.memset(amask[:], 0.0)
            nc.vector.memset(amask[:, START:START + 1], NEG)
            sel = res.tile([128, 1], dt.int32)
            nc.vector.memset(sel[:], START)

            fixedT = res.tile([128, 2, 128], F32)       # fixed2 transposed (feat, half, item)
            gv_res = res.tile([128, RN, D], F32)        # resident V nodes [item, node, feat]
            grawT = res.tile([128, 2, 128], F32)        # glimpse transposed (feat, half, item)
            gacc = res.tile([128, 2, 128], F32)         # glimpse accumulator per feat-half
            lgt = res.tile([128, N], F32)               # tanh'd logits
            lsb = res.tile([128, 2, 128], F32)          # logitsT staged from PSUM
            zl = res.tile([1, 128], dt.bfloat16)        # zero lhsT for PSUM bank clears
            nc.vector.memset(zl[:], 0.0)
            zr = res.tile([1, 512], dt.bfloat16)        # zero rhs for PSUM bank clears
            nc.vector.memset(zr[:], 0.0)

            # ---------- precompute A: projections ----------
            with (
                tc.tile_pool(name="wpool", bufs=1) as wp,
                tc.tile_pool(name="apool", bufs=2) as ap_,
                tc.tile_pool(name="bpool", bufs=1) as bp,
                tc.tile_pool(name="pmm", bufs=3, space="PSUM") as pmm,
            ):
                wn_sb = wp.tile([128, 2, 3 * D], F32)
                nc.sync.dma_start(wn_sb[:, 0, :], wnode_in[0:128, :])
                nc.sync.dma_start(wn_sb[:, 1, :], wnode_in[128:256, :])
                w2_sb = wp.tile([128, 2, D], F32)       # W_step[256:512]
                nc.sync.dma_start(w2_sb[:, 0, :], wstep_in[256:384, :])
                nc.sync.dma_start(w2_sb[:, 1, :], wstep_in[384:512, :])
                ws1_sb = wp.tile([128, 2, D], F32)      # W_step[0:256]
                nc.sync.dma_start(ws1_sb[:, 0, :], wstep_in[0:128, :])
                nc.sync.dma_start(ws1_sb[:, 1, :], wstep_in[128:256, :])
                wf_sb = wp.tile([128, 2, D], F32)
                nc.sync.dma_start(wf_sb[:, 0, :], wfix_in[0:128, :])
                nc.sync.dma_start(wf_sb[:, 1, :], wfix_in[128:256, :])
                wo_sb = wp.tile([128, 2, D], F32)
                nc.sync.dma_start(wo_sb[:, 0, :], wout_in[0:128, :])
                nc.sync.dma_start(wo_sb[:, 1, :], wout_in[128:256, :])
                pref_sb = wp.tile([128, D], F32)
                nc.sync.dma_start(
                    pref_sb[:],
                    pref_in.rearrange("(o f) -> o f", o=1).broadcast_to([128, D]),
                )

                # --- wl2 = W_node[:,512:768] @ W_out^T  (one-time fold) ---
                wnlT = wp.tile([128, 2, D], F32)        # [j-in-half, j-half, c]
                woT = wp.tile([128, 2, D], F32)         # [j-in-half, j-half, i]
                for kt in range(2):
                    for fh in range(2):
                        tp = psmall.tile([128, 512], F32, tag="tp")
                        nc.tensor.transpose(tp[:, 0:128], wn_sb[:, kt, 512 + fh * 128:512 + (fh + 1) * 128], ident[:])
                        nc.vector.tensor_copy(wnlT[:, fh, kt * 128:(kt + 1) * 128], tp[:, 0:128])
                        tp2 = psmall.tile([128, 512], F32, tag="tp")
                        nc.tensor.transpose(tp2[:, 0:128], wo_sb[:, kt, fh * 128:(fh + 1) * 128], ident[:])
                        nc.vector.tensor_copy(woT[:, fh, kt * 128:(kt + 1) * 128], tp2[:, 0:128])
                wl2_sb = wp.tile([128, 2, D], F32)      # [c-in-half, c-half, i]
                for mt in range(2):
                    pw_full = pmm.tile([128, 512], F32, tag="mm")
                    pw = pw_full[:, 0:D]
                    nc.tensor.matmul(pw[:], wnlT[:, 0, mt * 128:(mt + 1) * 128], woT[:, 0, :], start=True, stop=False)
                    nc.tensor.matmul(pw[:], wnlT[:, 1, mt * 128:(mt + 1) * 128], woT[:, 1, :], start=False, stop=True)
                    nc.vector.tensor_copy(wl2_sb[:, mt, :], pw[:])

                emb_rows = emb_in.rearrange("b n c -> (b n) c")

                def pre_body(rt):
                    erow = ap_.tile([128, D], F32, tag="erow")
                    nc.sync.dma_start(erow[:], emb_rows[bass.ds(rt * 128, 128), :])
                    e2 = ap_.tile([128, D], F32, tag="e2")
                    nc.vector.tensor_tensor(e2[:], erow[:], pref_sb[:], op=OP.add)
                    nc.sync.dma_start(emb2_d[bass.ds(rt * 128, 128), :], e2[:])
                    e2T = ap_.tile([128, 2, 128], F32, tag="e2T")
                    for ci in range(2):
                        tp = psmall.tile([128, 512], F32, tag="tp")
                        nc.tensor.transpose(tp[:, 0:128], e2[:, ci * 128:(ci + 1) * 128], ident[:])
                        nc.scalar.copy(e2T[:, ci, :], tp[:, 0:128])
                    # gV row-tile (natural layout)
                    pg_full = pmm.tile([128, 512], F32, tag="mm")
                    pg = pg_full[:, 0:D]
                    nc.tensor.matmul(pg[:], e2T[:, 0, :], wn_sb[:, 0, D:2 * D], start=True, stop=False)
                    nc.tensor.matmul(pg[:], e2T[:, 1, :], wn_sb[:, 1, D:2 * D], start=False, stop=True)
                    gvr = ap_.tile([128, D], F32, tag="gvr")
                    nc.vector.tensor_copy(gvr[:], pg[:])
                    nc.sync.dma_start(gv_d.rearrange("b n c -> (b n) c")[bass.ds(rt * 128, 128), :], gvr[:])
                    # transposed slabs: qdT (W2), gkT (Wn[:, :256]), lk2T (wl2)
                    for nm, wmat, dst in (("q", w2_sb, qdT_d), ("k", wn_sb, gkT_d), ("l", wl2_sb, lk2T_d)):
                        for mt in range(2):
                            pq_full = pmm.tile([128, 512], F32, tag="mm")
                            pq = pq_full[:, 0:128]
                            nc.tensor.matmul(pq[:], wmat[:, 0, mt * 128:(mt + 1) * 128], e2T[:, 0, :], start=True, stop=False)
                            nc.tensor.matmul(pq[:], wmat[:, 1, mt * 128:(mt + 1) * 128], e2T[:, 1, :], start=False, stop=True)
                            oq = ap_.tile([128, 128], F32, tag=f"o{nm}{mt}")
                            if nm == "q":
                                nc.scalar.copy(oq[:], pq[:])
                            else:
                                nc.vector.tensor_copy(oq[:], pq[:])
                            nc.sync.dma_start(dst[bass.ds(mt * 128, 128), bass.ds(rt * 128, 128)], oq[:])

                tc.For_i_unrolled(0, ROWT, 1, pre_body, max_unroll=2)

                # ---------- precompute B: fixed2 + fixedT ----------
                macc = bp.tile([128, D], F32, tag="macc")
                emb2_bnc = emb2_d.rearrange("(b n) c -> b n c", b=BS)
                for c in range(20):
                    ech = bp.tile([128, 10, D], F32, tag="ech")
                    nc.sync.dma_start(ech[:], emb2_bnc[:, c * 10:(c + 1) * 10, :])
                    part = bp.tile([128, D], F32, tag="mpart")
                    nc.vector.tensor_reduce(part[:], ech[:].transpose([0, 2, 1]), axis=AX.X, op=OP.add)
                    if c == 0:
                        nc.vector.tensor_copy(macc[:], part[:])
                    else:
                        nc.vector.tensor_tensor(macc[:], macc[:], part[:], op=OP.add)
                nc.vector.tensor_scalar(macc[:], macc[:], 1.0 / N, None, op0=OP.mult)
                first_sb = bp.tile([128, D], F32, tag="first")
                nc.sync.dma_start(first_sb[:], emb2_bnc[:, START, :])
                mT = bp.tile([128, 2, 128], F32, tag="mT")
                fT = bp.tile([128, 2, 128], F32, tag="fT")
                for ci in range(2):
                    tp = psmall.tile([128, 512], F32, tag="tp")
                    nc.tensor.transpose(tp[:, 0:128], macc[:, ci * 128:(ci + 1) * 128], ident[:])
                    nc.vector.tensor_copy(mT[:, ci, :], tp[:, 0:128])
                    tp2 = psmall.tile([128, 512], F32, tag="tp")
                    nc.tensor.transpose(tp2[:, 0:128], first_sb[:, ci * 128:(ci + 1) * 128], ident[:])
                    nc.vector.tensor_copy(fT[:, ci, :], tp2[:, 0:128])
                pf_full = pmm.tile([128, 512], F32, tag="mm")
                pf = pf_full[:, 0:D]
                nc.tensor.matmul(pf[:], mT[:, 0, :], wf_sb[:, 0, :], start=True, stop=False)
                nc.tensor.matmul(pf[:], mT[:, 1, :], wf_sb[:, 1, :], start=False, stop=False)
                nc.tensor.matmul(pf[:], fT[:, 0, :], ws1_sb[:, 0, :], start=False, stop=False)
                nc.tensor.matmul(pf[:], fT[:, 1, :], ws1_sb[:, 1, :], start=False, stop=True)
                fixed2 = bp.tile([128, D], F32, tag="fixed2")
                nc.vector.tensor_copy(fixed2[:], pf[:])
                for ci in range(2):
                    tp = psmall.tile([128, 512], F32, tag="tp")
                    nc.tensor.transpose(tp[:, 0:128], fixed2[:, ci * 128:(ci + 1) * 128], ident[:])
                    nc.vector.tensor_copy(fixedT[:, ci, :], tp[:, 0:128])

            # ---------- precompute C: S table ----------
            with (
                tc.tile_pool(name="cpool", bufs=2) as cp,
                tc.tile_pool(name="psb", bufs=1, space="PSUM") as psb,
            ):
                def s_body(b):
                    qt = cp.tile([128, 2, N], F32, tag="qt")
                    nc.sync.dma_start(qt[:, 0, :], qdT_d[0:128, bass.ds(b * N, N)])
                    nc.sync.dma_start(qt[:, 1, :], qdT_d[128:256, bass.ds(b * N, N)])
                    nc.vector.tensor_tensor(
                        qt[:], qt[:], fixedT[:, :, b:b + 1].broadcast_to([128, 2, N]), op=OP.add)
                    ktl = cp.tile([128, 2, N], F32, tag="ktl")
                    nc.sync.dma_start(ktl[:, 0, :], gkT_d[0:128, bass.ds(b * N, N)])
                    nc.sync.dma_start(ktl[:, 1, :], gkT_d[128:256, bass.ds(b * N, N)])
                    for mt, m0, msz in ((0, 0, 128), (1, 128, 72)):
                        stag = cp.tile([128, H, N], F32, tag=f"stag{mt}")
                        for hg in range(2):
                            pp = psb.tile([128, 4, 512], F32, tag="sp")
                            for hh in range(4):
                                h = hg * 4 + hh
                                ktile = h // 4
                                krow = (h % 4) * 32
                                nc.tensor.matmul(
                                    pp[0:msz, hh, 0:N],
                                    qt[krow:krow + 32, ktile, m0:m0 + msz],
                                    ktl[krow:krow + 32, ktile, :],
                                    start=True, stop=True,
                                    tile_position=(krow, 0))
                            if hg == 0:
                                nc.vector.tensor_scalar(
                                    stag[0:msz, 0:4, :], pp[0:msz, 0:4, 0:N], ISD32, None, op0=OP.mult)
                            else:
                                nc.scalar.activation(
                                    stag[0:msz, 4:8, :], pp[0:msz, 0:4, 0:N], ACTF.Copy, scale=ISD32)
                        nc.sync.dma_start(
                            S_d[bass.ds(b * N + m0, msz), :],
                            stag[0:msz, :, :].rearrange("p h n -> p (h n)"))

                for b in range(BS):
                    s_body(b)

            # ---------- resident V fill ----------
            nc.sync.dma_start(gv_res[:], gv_d[:, 0:RN, :])

            # ---------- decode steps ----------
            with (
                tc.tile_pool(name="srow", bufs=2) as srp,
                tc.tile_pool(name="gvs", bufs=2) as gvsp,
                tc.tile_pool(name="prp", bufs=2) as prp,
                tc.tile_pool(name="lkp", bufs=4) as lkp,
                tc.tile_pool(name="wk", bufs=2) as wk,
                tc.tile_pool(name="ppl", bufs=2, space="PSUM") as ppl,
            ):
                def step_body(s):
                    # --- gather S row for (b, prev) ---
                    offs = wk.tile([128, 1], dt.int32, tag="offs")
                    nc.vector.tensor_tensor(offs[:], iota_row[:], sel[:], op=OP.add)
                    srow = srp.tile([128, H, N], F32, tag="srow")
                    nc.gpsimd.indirect_dma_start(
                        out=srow[:].rearrange("p h n -> p (h n)"), out_offset=None,
                        in_=S_d, in_offset=IndirectOffsetOnAxis(ap=offs[:], axis=0))
                    # --- masked softmax per head (in-place in srow) ---
                    ab = amask[:].rearrange("p (o n) -> p o n", o=1).broadcast_to([128, H, N])
                    nc.vector.tensor_tensor(srow[:], srow[:], ab, op=OP.add)
                    mh = wk.tile([128, H], F32, tag="mh")
                    nc.vector.tensor_reduce(mh[:], srow[:], axis=AX.X, op=OP.max)
                    nc.vector.tensor_tensor(
                        srow[:], srow[:],
                        mh[:].rearrange("p (h o) -> p h o", o=1).broadcast_to([128, H, N]),
                        op=OP.subtract)
                    nc.scalar.activation(srow[:], srow[:], ACTF.Exp)
                    sh = wk.tile([128, H], F32, tag="sh")
                    nc.vector.tensor_reduce(sh[:], srow[:], axis=AX.X, op=OP.add)
                    rh = wk.tile([128, H], F32, tag="rh")
                    nc.vector.reciprocal(rh[:], sh[:])
                    # srow now holds unnormalized attn (exp); divide by Z after glimpse.

                    pp = ppl.tile([128, 2, 512], F32, tag="pp")
                    for mt in range(2):
                        nc.tensor.matmul(pp[:, mt, :], zl[:], zr[:], start=True, stop=True,
                                         skip_group_check=True)
                    # --- glimpse + logits, per feature-half ---
                    for ft in range(2):
                        f0 = ft * 128
                        nch = N // CH
                        for c in range(nch):
                            n0 = c * CH
                            if n0 + CH <= RN:
                                src = gv_res[:, n0:n0 + CH, f0:f0 + 128]
                            else:
                                gvs = gvsp.tile([128, CH, 128], F32, tag="gvs")
                                dql = nc.sync if (c % 2 == 0) else nc.scalar
                                dql.dma_start(gvs[:], gv_d[:, n0:n0 + CH, f0:f0 + 128])
                                src = gvs[:]
                            av = srow[:, ft * 4:(ft + 1) * 4, n0:n0 + CH].transpose([0, 2, 1]) \
                                .rearrange("p n (h o) -> p n h o", o=1).broadcast_to([128, CH, 4, d])
                            pr = prp.tile([128, CH, 128], F32, tag="pr")
                            eng = nc.gpsimd if (c % 4) != 3 else nc.vector
                            eng.tensor_tensor(
                                pr[:].rearrange("p n (h e) -> p n h e", h=4),
                                src.rearrange("p n (h e) -> p n h e", h=4), av, op=OP.mult)
                            gpart = wk.tile([128, 128], F32, tag="gpart")
                            nc.vector.tensor_reduce(
                                gpart[:], pr[:].transpose([0, 2, 1]), axis=AX.X, op=OP.add)
                            if c == 0:
                                nc.vector.tensor_copy(gacc[:, ft, :], gpart[:])
                            else:
                                nc.vector.tensor_tensor(gacc[:, ft, :], gacc[:, ft, :], gpart[:], op=OP.add)
                        # normalize by 1/Z per head
                        rv = rh[:, ft * 4:(ft + 1) * 4].rearrange("p (h o) -> p h o", o=1) \
                            .broadcast_to([128, 4, d])
                        nc.gpsimd.tensor_tensor(
                            gacc[:, ft, :].rearrange("p (h e) -> p h e", h=4),
                            gacc[:, ft, :].rearrange("p (h e) -> p h e", h=4), rv, op=OP.mult)
                        # transpose glimpse half -> grawT[:, ft, :]
                        tg = psmall.tile([128, 512], F32, tag="tp")
                        nc.tensor.transpose(tg[:, 0:128], gacc[:, ft, :], ident[:])
                        nc.scalar.copy(grawT[:, ft, :], tg[:, 0:128])
                        # logits MMs for this feature-half (accumulate over ft)
                        for icb in range(BS // IC):
                            b0 = icb * IC
                            lkb = lkp.tile([128, IC, N], F32, tag="lkb")
                            dqb = nc.sync if (icb % 2 == 0) else nc.scalar
                            dqb.dma_start(
                                lkb[:].rearrange("p i n -> p (i n)"),
                                lk2T_d[bass.ds(f0, 128), bass.ds(b0 * N, IC * N)])
                            for i in range(IC):
                                bb = b0 + i
                                for mt, m0, msz in ((0, 0, 128), (1, 128, 72)):
                                    nc.tensor.matmul(
                                        pp[0:msz, mt, bb:bb + 1],
                                        lkb[:, i, m0:m0 + msz],
                                        grawT[:, ft, bb:bb + 1],
                                        start=False, stop=True,
                                        skip_group_check=True)
                    # --- retranspose logits + tanh ---
                    nc.vector.tensor_copy(lsb[:], pp[:, :, 0:128])
                    for mt, m0, msz in ((0, 0, 128), (1, 128, 72)):
                        tl = psmall.tile([128, 512], F32, tag="tp")
                        nc.tensor.transpose(tl[:, 0:128], lsb[:, mt, :], ident[:])
                        nc.scalar.activation(lgt[:, m0:m0 + msz], tl[:, 0:msz], ACTF.Tanh, scale=ISD256)
                    logits = wk.tile([128, N], F32, tag="logits")
                    nc.gpsimd.tensor_scalar(logits[:], lgt[:], 10.0, None, op0=OP.mult)
                    nc.vector.tensor_tensor(logits[:], logits[:], amask[:], op=OP.add)
                    # --- argmax ---
                    mx8 = wk.tile([128, 8], F32, tag="mx8")
                    nc.vector.max(mx8[:], logits[:])
                    ix8 = wk.tile([128, 8], dt.uint32, tag="ix8")
                    nc.vector.max_index(ix8[:], mx8[:], logits[:])
                    nc.vector.tensor_copy(sel[:], ix8[:, 0:1])
                    # --- log_softmax + output ---
                    m1 = wk.tile([128, 1], F32, tag="m1")
                    nc.vector.tensor_reduce(m1[:], logits[:], axis=AX.X, op=OP.max)
                    shl = wk.tile([128, N], F32, tag="shl")
                    nc.gpsimd.tensor_tensor(shl[:], logits[:], m1[:].broadcast_to([128, N]), op=OP.subtract)
                    pexp = wk.tile([128, N], F32, tag="pexp")
                    s1 = wk.tile([128, 1], F32, tag="s1")
                    nc.scalar.activation(pexp[:], shl[:], ACTF.Exp, accum_out=s1[:])
                    ls = wk.tile([128, 1], F32, tag="ls")
                    nc.scalar.activation(ls[:], s1[:], ACTF.Ln)
                    lp = wk.tile([128, N], F32, tag="lp")
                    nc.gpsimd.tensor_tensor(lp[:], shl[:], ls[:].broadcast_to([128, N]), op=OP.subtract)
                    nc.scalar.dma_start(out[:, bass.ds(s * N, N)], lp[:])
                    # --- visited mask update ---
                    ohf = wk.tile([128, N], F32, tag="ohf")
                    nc.vector.tensor_tensor(ohf[:], iota_n[:], sel[:].broadcast_to([128, N]), op=OP.is_equal)
                    nc.gpsimd.tensor_scalar(ohf[:], ohf[:], NEG, None, op0=OP.mult)
                    nc.vector.tensor_tensor(amask[:], amask[:], ohf[:], op=OP.add)

                tc.For_i_unrolled(0, T, 1, step_body, max_unroll=4)

    nc.compile()
    return nc


_CACHE = {}


def kernel(**inputs) -> np.ndarray:
    if "nc" not in _CACHE:
        _CACHE["nc"] = _build()
    nc = _CACHE["nc"]

    emb = np.ascontiguousarray(np.asarray(inputs["embeddings"], np.float32))
    shared = {
        "pref_embed": np.asarray(inputs["pref_embed"], np.float32),
        "W_node": np.asarray(inputs["W_node"], np.float32),
        "W_fixed": np.asarray(inputs["W_fixed"], np.float32),
        "W_step": np.asarray(inputs["W_step"], np.float32),
        "W_out": np.asarray(inputs["W_out"], np.float32),
    }
    in_maps = []
    for i in range(NCORES):
        m = {"embeddings": emb[i * BS:(i + 1) * BS]}
        m.update(shared)
        in_maps.append(m)

    res = run_bass_kernel_spmd(nc, in_maps, list(range(NCORES)))
    outs = [res.results[i]["log_p"].reshape(BS, T, N) for i in range(NCORES)]
    return np.concatenate(outs, axis=0)


if __name__ == "__main__":
    z = np.load("inputs.npz")
    inp = {k: z[k] for k in z.files}
    o = kernel(**inp)
    print("kernel output", o.shape, o.dtype)
    np.save("kernel_out.npy", o)
